# revision 39
# baseline (speedup 1.0000x reference)
"""Trainium2 Bass kernel for the LSTM trajectory decoder.

Strategy: data-parallel over batch (B=512 -> 64 per core on 8 cores).
All weights replicated and resident in SBUF. The sequential T=200 LSTM
recurrence runs per-core with:
  - gates/hidden matmuls with fp16 operands + fp32 PSUM accumulate
    (fp16 streams at full PE rate; values here are all < 10 so fp16's
    5-bit exponent is safe and its 10-bit mantissa keeps rel err ~1e-3)
  - batch-major layout [64, feat] for gates/elementwise (activations are
    the PE stationary operand, weights stream as the moving operand)
  - PE transposes to produce feature-major activations (h^T, a1^T, o1^T)
    needed as stationary operands / small-matmul inputs
  - outputs accumulated on-chip feature-major ([7+3, T*64]) and
    transposed on the host (free), FK projection done on-device at the end.
"""

import numpy as np

B, T = 512, 200
LAT, JD, OD, H = 128, 7, 3, 512
NL = 8
FK = NL * 3  # 24
NCORES = 8
BL = B // NCORES  # 64 batch per core
G4 = 4 * H  # 2048
TB = T * BL  # 12800 free-dim length of output buffers

_CACHE = {}


def _round_f32r(a):
    """Round an fp32 array to fp32r (11-bit mantissa) on the host so the
    values we feed match what the PE consumes."""
    a = np.ascontiguousarray(a, dtype=np.float32)
    bits = a.view(np.uint32)
    # round-to-nearest-even on the low 13 bits
    rounded = (bits + 0x0FFF + ((bits >> 13) & 1)) & 0xFFFFE000
    return rounded.view(np.float32)


def _build(nsteps=T):
    import concourse.bass as bass
    import concourse.tile as tile
    from concourse import bacc, mybir

    F32 = mybir.dt.float32
    F16 = mybir.dt.float16
    AF = mybir.ActivationFunctionType
    ALU = mybir.AluOpType
    ts = bass.ts

    nc = bacc.Bacc("TRN2", target_bir_lowering=False, debug=False)

    def din(name, shape, dt=F16):
        return nc.dram_tensor(name, list(shape), dt, kind="ExternalInput").ap()

    # --- DRAM inputs (per-core layouts prepared on host) ---
    zT_d = din("zT", [LAT, BL])                     # z slice, transposed
    WihzT_d = din("WihzT", [LAT, G4])               # W_ih[:, 7:].T
    WihpTa_d = din("WihpTa", [JD + 1, G4])          # rows 0-6 W_ih[:, :7].T, row 7 = b_ih+b_hh
    WhhT_d = din("WhhT", [128, 4, G4])              # W_hh.T chunked on K
    Wa1T_d = din("Wa1T", [128, 4, H])
    Wo1T_d = din("Wo1T", [128, 4, H])
    Wa2T_d = din("Wa2T", [128, 4, JD])
    Wo2T_d = din("Wo2T", [128, 4, OD])
    WzhT_d = din("WzhT", [LAT, H])
    WzcT_d = din("WzcT", [LAT, H])
    bzh_d = din("bzh", [128, 4], F32)               # per-partition bias for h0^T chunks
    bzc_d = din("bzc", [1, H])                      # ones-matmul row for c0
    ba1_d = din("ba1", [1, H])
    bo1_d = din("bo1", [1, H])
    ba2_d = din("ba2", [JD, 1], F32)
    bo2_d = din("bo2", [OD, 1], F32)
    WfkT_d = din("WfkT", [JD, FK], F32)
    bfk_d = din("bfk", [FK, 1], F32)
    jrange_d = din("jrange", [JD, 1], F32)
    jmean_d = din("jmean", [JD, 1], F32)
    prevTa_d = din("prevTa0", [JD + 1, BL])         # initial [prev0^T; ones]
    ones_d = din("ones", [1, BL])
    iden_d = din("iden", [128, 128], F32)

    TBn = nsteps * BL
    combT_d = nc.dram_tensor("combT", [FK + OD, TBn], F32, kind="ExternalOutput").ap()
    jointT_d = nc.dram_tensor("jointT", [JD, TBn], F32, kind="ExternalOutput").ap()

    with tile.TileContext(nc) as tc:
        with (
            tc.tile_pool(name="consts", bufs=1) as consts,
            tc.tile_pool(name="state", bufs=1) as state,
            tc.tile_pool(name="acts", bufs=8) as acts,
            tc.tile_pool(name="psG", bufs=4, space="PSUM") as psG,
            tc.tile_pool(name="psS", bufs=1, space="PSUM") as psS,
        ):
            def load(dram, shape, dt=F16):
                t = consts.tile(list(shape), dt, tag=dram.tensor.name)
                nc.sync.dma_start(t[:], dram[:])
                return t

            zT = load(zT_d, [LAT, BL])
            WihzT = load(WihzT_d, [LAT, G4])
            WihpTa = load(WihpTa_d, [JD + 1, G4])
            WhhT = load(WhhT_d, [128, 4, G4])
            Wa1T = load(Wa1T_d, [128, 4, H])
            Wo1T = load(Wo1T_d, [128, 4, H])
            Wa2T = load(Wa2T_d, [128, 4, JD])
            Wo2T = load(Wo2T_d, [128, 4, OD])
            WzhT = load(WzhT_d, [LAT, H])
            WzcT = load(WzcT_d, [LAT, H])
            bzh = load(bzh_d, [128, 4], F32)
            bzc = load(bzc_d, [1, H])
            ba1 = load(ba1_d, [1, H])
            bo1 = load(bo1_d, [1, H])
            ba2 = load(ba2_d, [JD, 1], F32)
            bo2 = load(bo2_d, [OD, 1], F32)
            WfkT = load(WfkT_d, [JD, FK], F32)
            bfk = load(bfk_d, [FK, 1], F32)
            jrange = load(jrange_d, [JD, 1], F32)
            jmean = load(jmean_d, [JD, 1], F32)
            ones = load(ones_d, [1, BL])
            iden = load(iden_d, [128, 128], F32)

            prevTa = state.tile([JD + 1, BL], F16)
            nc.sync.dma_start(prevTa[:], prevTa_d[:])
            hT = state.tile([128, 4, BL], F16)
            c = state.tile([BL, H], F32)
            h = state.tile([BL, H], F32)
            a1T = state.tile([128, 4, BL], F16)
            o1T = state.tile([128, 4, BL], F16)
            outT = state.tile([32 + OD, TBn], F32)  # rows 0:7 joints^T, rows 32:35 obj^T (32-aligned partition base)

            idq = iden[0:BL, 0:BL]  # 64x64 identity for transposes

            # ---- static PSUM tiles (4 banks; psG holds the other 4) ----
            # tph bank: h-transposes (cols 0:256) + 4 raw-head partial matmuls
            # (cols 256:512) — all single-write matmul groups (start+stop on
            # every instruction), so they can safely share a zero region.
            # tpa bank: a1- AND o1-transposes time-share cols 0:256 (WAR deps
            # order them), obj-head partials on cols 256:512.
            tph = psS.tile([128, H], F32)
            tpa = psS.tile([128, H], F32)
            c0p = psS.tile([BL, H], F32)   # init-only scratch (1 bank)
            rawp = tph[0:JD, 4 * BL : 8 * BL]     # [7, 4*64] partials
            objp = tpa[0:OD, 4 * BL : 8 * BL]     # [3, 4*64] partials

            # ---- init: h0^T (feature-major) and c0 (batch-major) ----
            for m in range(4):
                nc.tensor.matmul(tph[:, ts(m, BL)], WzhT[:, ts(m, 128)], zT[:], start=True, stop=True)
                nc.scalar.activation(hT[:, m, :], tph[:, ts(m, BL)], AF.Identity, bias=bzh[:, m : m + 1])
            nc.tensor.matmul(c0p[:], zT[:], WzcT[:], start=True, stop=False)
            nc.tensor.matmul(c0p[:], ones[:], bzc[:], start=False, stop=True)
            nc.vector.tensor_copy(c[:], c0p[:])

            # ---- recurrence ----
            # Gate banks are partition-packed in pairs: tile gA holds gate f on
            # partitions 0:64 and gate i on 64:128 (so one sigmoid covers both
            # and each tensor_tensor's SBUF inputs share a base partition);
            # gB holds g / o. Upper-half matmuls use PE column group (0,64).
            # Gate pairs partition-packed per PSUM tile: gA = f on partitions
            # 0:64 + i on 64:128 (one sigmoid covers both; f at base 0 pairs
            # with c, i at base 64 pairs with tanh(g) written at base 64);
            # gB = g top / o bottom. Upper halves use PE column group (0,64).
            PAIRS = ((1, 0), (2, 3))

            def z_mms(gp):
                for half, (bt, bb) in enumerate(PAIRS):
                    nc.tensor.matmul(gp[half][0:BL, :], zT[:], WihzT[:, ts(bt, H)], start=True, stop=False, skip_group_check=True)
                    nc.tensor.matmul(gp[half][BL : 2 * BL, :], zT[:], WihzT[:, ts(bb, H)], start=True, stop=False, skip_group_check=True)

            def h_mms(gp, ks):
                for k in ks:
                    lhs = hT[:, k, :]
                    for half, (bt, bb) in enumerate(PAIRS):
                        nc.tensor.matmul(gp[half][0:BL, :], lhs, WhhT[:, k, ts(bt, H)], start=False, stop=False, skip_group_check=True)
                        nc.tensor.matmul(gp[half][BL : 2 * BL, :], lhs, WhhT[:, k, ts(bb, H)], start=False, stop=False, skip_group_check=True)

            def prev_mms(gp):
                for half, (bt, bb) in enumerate(PAIRS):
                    nc.tensor.matmul(gp[half][0:BL, :], prevTa[:], WihpTa[:, ts(bt, H)], start=False, stop=True, skip_group_check=True)
                    nc.tensor.matmul(gp[half][BL : 2 * BL, :], prevTa[:], WihpTa[:, ts(bb, H)], start=False, stop=True, skip_group_check=True)

            def alloc_g(t):
                return [psG.tile([2 * BL, H], F32, tag="g", name=f"g{half}_{t}") for half in range(2)]

            g_cur = alloc_g(0)
            z_mms(g_cur)
            h_mms(g_cur, range(4))
            prev_mms(g_cur)

            for t in range(nsteps):
                last = t + 1 >= nsteps
                if not last:
                    g_next = alloc_g(t + 1)
                    z_mms(g_next)  # PE fills the elementwise window below

                # LSTM cell elementwise (gate order i, f, g, o), processed in
                # two H-halves so the next step's gate matmuls for k=0,1 can
                # start while the second half is still on ACT/DVE.
                gA, gB = g_cur
                sif = acts.tile([2 * BL, H], F32, tag="actw")   # sigmoid(f) top, sigmoid(i) bottom
                tgt = acts.tile([2 * BL, H], F32, tag="actw2")  # tanh(g) on bottom half (base 64, pairs with si)
                so = acts.tile([BL, H], F32, tag="act")
                t1 = acts.tile([BL, H], F32, tag="act")
                t2 = acts.tile([BL, H], F32, tag="act")
                tch = acts.tile([BL, H], F32, tag="act")
                NSPLIT = 2
                HH = H // NSPLIT
                for hh in range(NSPLIT):
                    sl = bass.ds(hh * HH, HH)
                    nc.scalar.activation(sif[:, sl], gA[0 : 2 * BL, sl], AF.Sigmoid)
                    nc.scalar.activation(tgt[BL : 2 * BL, sl], gB[0:BL, sl], AF.Tanh)
                    nc.scalar.activation(so[:, sl], gB[BL : 2 * BL, sl], AF.Sigmoid)
                    nc.gpsimd.tensor_mul(t1[:, sl], sif[0:BL, sl], c[:, sl])
                    nc.vector.tensor_mul(t2[:, sl], sif[BL : 2 * BL, sl], tgt[BL : 2 * BL, sl])
                    nc.vector.tensor_add(c[:, sl], t1[:, sl], t2[:, sl])
                    nc.scalar.activation(tch[:, sl], c[:, sl], AF.Tanh)
                    nc.vector.tensor_mul(h[:, sl], so[:, sl], tch[:, sl])
                    ks = range(4 // NSPLIT * hh, 4 // NSPLIT * (hh + 1))
                    for k in ks:
                        nc.tensor.transpose(tph[:, ts(k, BL)], h[:, ts(k, 128)], idq)
                    nc.vector.tensor_copy(
                        hT[:, ks.start : ks.stop, :].rearrange("p a b -> p (a b)"),
                        tph[:, ks.start * BL : ks.stop * BL],
                    )
                    if not last:
                        h_mms(g_next, ks)

                # head first layers, packed: a1 on partitions 0:64, o1 on 64:128
                m1 = psG.tile([2 * BL, H], F32, tag="m1", bufs=1, name=f"m1_{t}")
                for k in range(4):
                    nc.tensor.matmul(m1[0:BL, :], hT[:, k, :], Wa1T[:, k, :], start=(k == 0), stop=False, skip_group_check=True)
                    nc.tensor.matmul(m1[BL : 2 * BL, :], hT[:, k, :], Wo1T[:, k, :], start=(k == 0), stop=False, skip_group_check=True)
                nc.tensor.matmul(m1[0:BL, :], ones[:], ba1[:], start=False, stop=True, skip_group_check=True)
                nc.tensor.matmul(m1[BL : 2 * BL, :], ones[:], bo1[:], start=False, stop=True, skip_group_check=True)

                # agent head: raw^T = tanh(Wa2 @ relu(a1)^T + ba2)
                # Wa2 contraction is done as 4 side-by-side single-matmul
                # partials + one strided reduce (keeps shared banks free of
                # multi-matmul accumulation groups).
                a1 = acts.tile([BL, H], F32, tag="act")
                nc.scalar.activation(a1[:], m1[0:BL, :], AF.Relu)
                for k in range(4):
                    nc.tensor.transpose(tpa[:, ts(k, BL)], a1[:, ts(k, 128)], idq)
                nc.vector.tensor_copy(a1T[:].rearrange("p a b -> p (a b)"), tpa[:, 0 : 4 * BL])
                for k in range(4):
                    nc.tensor.matmul(rawp[:, ts(k, BL)], Wa2T[:, k, :], a1T[:, k, :], start=True, stop=True)
                raw_red = acts.tile([JD, BL], F32, tag="red", name=f"rr{t}")
                nc.vector.reduce_sum(
                    raw_red[:], rawp.rearrange("p (a b) -> p b a", a=4), axis=mybir.AxisListType.X
                )
                rawt = acts.tile([JD, BL], F32, tag="red", name=f"rt{t}")
                nc.scalar.activation(rawt[:], raw_red[:], AF.Tanh, bias=ba2[:])
                nc.vector.tensor_copy(prevTa[0:JD, :], rawt[:])
                nc.vector.tensor_scalar(
                    outT[0:JD, ts(t, BL)], rawt[:], jrange[:], jmean[:],
                    op0=ALU.mult, op1=ALU.add,
                )

                if not last:
                    prev_mms(g_next)

                # object head — entirely off the critical path; its PE ops run
                # inside the next step's elementwise window. o1-transposes
                # time-share tpa cols 0:256 with the a1-transposes (WAR-ordered).
                o1 = acts.tile([BL, H], F32, tag="act")
                nc.scalar.activation(o1[:], m1[BL : 2 * BL, :], AF.Relu)
                for k in range(4):
                    nc.tensor.transpose(tpa[:, ts(k, BL)], o1[:, ts(k, 128)], idq)
                nc.vector.tensor_copy(o1T[:].rearrange("p a b -> p (a b)"), tpa[:, 0 : 4 * BL])
                for k in range(4):
                    nc.tensor.matmul(objp[:, ts(k, BL)], Wo2T[:, k, :], o1T[:, k, :], start=True, stop=True)
                obj_red = acts.tile([OD, BL], F32, tag="red2", name=f"ob{t}")
                nc.vector.reduce_sum(
                    obj_red[:], objp.rearrange("p (a b) -> p b a", a=4), axis=mybir.AxisListType.X
                )
                nc.scalar.activation(outT[32 : 32 + OD, ts(t, BL)], obj_red[:], AF.Identity, bias=bo2[:])

                if not last:
                    g_cur = g_next

            # ---- FK projection + output DMAs ----
            for nb in range(max(1, TBn // 512)):
                fkp = c0p[0:FK, :]
                fkw = min(512, TBn - nb * 512)
                nc.tensor.matmul(fkp[:, 0:fkw], WfkT[:], outT[0:JD, nb*512:nb*512+fkw], start=True, stop=True)
                ag = acts.tile([FK, 512], F32, tag="act", name=f"ag{nb}")
                nc.scalar.activation(ag[:, 0:fkw], fkp[:, 0:fkw], AF.Identity, bias=bfk[:])
                nc.sync.dma_start(combT_d[0:FK, nb*512:nb*512+fkw], ag[:, 0:fkw])
            nc.sync.dma_start(combT_d[FK : FK + OD, :], outT[32 : 32 + OD, :])
            nc.sync.dma_start(jointT_d[:], outT[0:JD, :])

    nc.compile()
    return nc


def _prep_in_maps(inputs):
    import ml_dtypes
    f32 = lambda a: np.ascontiguousarray(np.asarray(a), dtype=np.float32)
    r = lambda a: np.ascontiguousarray(np.asarray(a, np.float32), dtype=np.float16)

    z = f32(inputs["z"])
    W_ih = f32(inputs["W_ih"])
    W_hh = f32(inputs["W_hh"])
    b_ih = f32(inputs["b_ih"])
    b_hh = f32(inputs["b_hh"])
    W_zh, b_zh = f32(inputs["W_zh"]), f32(inputs["b_zh"])
    W_zc, b_zc = f32(inputs["W_zc"]), f32(inputs["b_zc"])
    Wo1, bo1 = f32(inputs["Wo1"]), f32(inputs["bo1"])
    Wo2, bo2 = f32(inputs["Wo2"]), f32(inputs["bo2"])
    Wa1, ba1 = f32(inputs["Wa1"]), f32(inputs["ba1"])
    Wa2, ba2 = f32(inputs["Wa2"]), f32(inputs["ba2"])
    start_token = f32(inputs["start_token"])
    jl, ju = f32(inputs["joint_lower"]), f32(inputs["joint_upper"])
    W_fk, b_fk = f32(inputs["W_fk"]), f32(inputs["b_fk"])

    jrange = (ju - jl) * 0.5
    jmean = (ju + jl) * 0.5
    prev0 = (start_token - jmean) / jrange

    WihzT = r(W_ih[:, JD:].T)                                   # [128, 2048]
    WihpTa = r(np.concatenate([W_ih[:, :JD].T, (b_ih + b_hh)[None, :]], axis=0))  # [8, 2048]
    WhhT = r(W_hh.T.reshape(4, 128, G4).transpose(1, 0, 2))     # [128, 4, 2048]
    Wa1T = r(Wa1.T.reshape(4, 128, H).transpose(1, 0, 2))
    Wo1T = r(Wo1.T.reshape(4, 128, H).transpose(1, 0, 2))
    Wa2T = r(Wa2.T.reshape(4, 128, JD).transpose(1, 0, 2))
    Wo2T = r(Wo2.T.reshape(4, 128, OD).transpose(1, 0, 2))
    WzhT, WzcT = r(W_zh.T), r(W_zc.T)
    bzh = np.ascontiguousarray(b_zh.reshape(4, 128).T)          # [128, 4]
    shared = {
        "WihzT": WihzT, "WihpTa": WihpTa, "WhhT": WhhT,
        "Wa1T": Wa1T, "Wo1T": Wo1T, "Wa2T": Wa2T, "Wo2T": Wo2T,
        "WzhT": WzhT, "WzcT": WzcT, "bzh": bzh,
        "bzc": r(b_zc[None, :]), "ba1": r(ba1[None, :]), "bo1": r(bo1[None, :]),
        "ba2": ba2[:, None], "bo2": bo2[:, None],
        "WfkT": np.ascontiguousarray(W_fk.T), "bfk": b_fk[:, None],
        "jrange": jrange[:, None], "jmean": jmean[:, None],
        "ones": r(np.ones((1, BL), np.float32)),
        "iden": np.eye(128, dtype=np.float32),
    }
    prevTa0 = r(np.concatenate(
        [np.broadcast_to(prev0[:, None], (JD, BL)), np.ones((1, BL), np.float32)], axis=0))

    in_maps = []
    for cid in range(NCORES):
        zs = z[cid * BL : (cid + 1) * BL]
        m = dict(shared)
        m["zT"] = r(zs.T)
        m["prevTa0"] = prevTa0
        in_maps.append(m)
    return in_maps


def _get_compiled():
    if "nc" not in _CACHE:
        _CACHE["nc"] = _build()
    return _CACHE["nc"]


def run_on_cores(inputs, trace=False, **kw):
    """Compile (cached) + execute; returns (combined, joint, BassKernelResults)."""
    from concourse.bass_utils import run_bass_kernel_spmd

    nc = _get_compiled()
    in_maps = _prep_in_maps(inputs)
    res = run_bass_kernel_spmd(nc, in_maps, list(range(NCORES)), trace=trace, **kw)
    combined = np.empty((B, T, FK + OD), np.float32)
    joint = np.empty((B, T, JD), np.float32)
    for cid in range(NCORES):
        combT = res.results[cid]["combT"]  # [27, T*BL]
        jointT = res.results[cid]["jointT"]  # [7, T*BL]
        sl = slice(cid * BL, (cid + 1) * BL)
        combined[sl] = combT.reshape(FK + OD, T, BL).transpose(2, 1, 0)
        joint[sl] = jointT.reshape(JD, T, BL).transpose(2, 1, 0)
    return combined, joint, res


def kernel(**inputs):
    combined, joint, _ = run_on_cores(inputs)
    return combined, joint


# revision 41
# speedup vs baseline: 1224.0917x; 1224.0917x over previous
"""Trainium2 Bass kernel for the LSTM trajectory decoder.

Strategy: data-parallel over batch (B=512 -> 64 per core on 8 cores).
All weights replicated and resident in SBUF. The sequential T=200 LSTM
recurrence runs per-core with:
  - gates/hidden matmuls with fp16 operands + fp32 PSUM accumulate
    (fp16 streams at full PE rate; values here are all < 10 so fp16's
    5-bit exponent is safe and its 10-bit mantissa keeps rel err ~1e-3)
  - batch-major layout [64, feat] for gates/elementwise (activations are
    the PE stationary operand, weights stream as the moving operand)
  - PE transposes to produce feature-major activations (h^T, a1^T, o1^T)
    needed as stationary operands / small-matmul inputs
  - outputs accumulated on-chip feature-major ([7+3, T*64]) and
    transposed on the host (free), FK projection done on-device at the end.
"""

import numpy as np

B, T = 512, 200
LAT, JD, OD, H = 128, 7, 3, 512
NL = 8
FK = NL * 3  # 24
NCORES = 8
BL = B // NCORES  # 64 batch per core
G4 = 4 * H  # 2048
TB = T * BL  # 12800 free-dim length of output buffers

_CACHE = {}


def _round_f32r(a):
    """Round an fp32 array to fp32r (11-bit mantissa) on the host so the
    values we feed match what the PE consumes."""
    a = np.ascontiguousarray(a, dtype=np.float32)
    bits = a.view(np.uint32)
    # round-to-nearest-even on the low 13 bits
    rounded = (bits + 0x0FFF + ((bits >> 13) & 1)) & 0xFFFFE000
    return rounded.view(np.float32)


def _build(nsteps=T):
    import concourse.bass as bass
    import concourse.tile as tile
    from concourse import bacc, mybir

    F32 = mybir.dt.float32
    F16 = mybir.dt.float16
    AF = mybir.ActivationFunctionType
    ALU = mybir.AluOpType
    ts = bass.ts

    nc = bacc.Bacc("TRN2", target_bir_lowering=False, debug=False)

    def din(name, shape, dt=F16):
        return nc.dram_tensor(name, list(shape), dt, kind="ExternalInput").ap()

    # --- DRAM inputs (per-core layouts prepared on host) ---
    zT_d = din("zT", [LAT, BL])                     # z slice, transposed
    WihzT_d = din("WihzT", [LAT, G4])               # W_ih[:, 7:].T
    WihpTa_d = din("WihpTa", [JD + 1, G4])          # rows 0-6 W_ih[:, :7].T, row 7 = b_ih+b_hh
    WhhT_d = din("WhhT", [128, 4, G4])              # W_hh.T chunked on K
    Wa1T_d = din("Wa1T", [128, 4, H])
    Wo1T_d = din("Wo1T", [128, 4, H])
    Wa2T_d = din("Wa2T", [128, 4, JD])
    Wo2T_d = din("Wo2T", [128, 4, OD])
    WzhT_d = din("WzhT", [LAT, H])
    WzcT_d = din("WzcT", [LAT, H])
    bzh_d = din("bzh", [128, 4], F32)               # per-partition bias for h0^T chunks
    bzc_d = din("bzc", [1, H])                      # ones-matmul row for c0
    ba1_d = din("ba1", [1, H])
    bo1_d = din("bo1", [1, H])
    ba2_d = din("ba2", [JD, 1], F32)
    bo2_d = din("bo2", [OD, 1], F32)
    WfkT_d = din("WfkT", [JD, FK], F32)
    bfk_d = din("bfk", [FK, 1], F32)
    jrange_d = din("jrange", [JD, 1], F32)
    jmean_d = din("jmean", [JD, 1], F32)
    prevTa_d = din("prevTa0", [JD + 1, BL])         # initial [prev0^T; ones]
    ones_d = din("ones", [1, BL])
    iden_d = din("iden", [128, 128], F32)

    TBn = nsteps * BL
    combT_d = nc.dram_tensor("combT", [FK + OD, TBn], F32, kind="ExternalOutput").ap()
    jointT_d = nc.dram_tensor("jointT", [JD, TBn], F32, kind="ExternalOutput").ap()

    with tile.TileContext(nc) as tc:
        with (
            tc.tile_pool(name="consts", bufs=1) as consts,
            tc.tile_pool(name="state", bufs=1) as state,
            tc.tile_pool(name="acts", bufs=8) as acts,
            tc.tile_pool(name="psG", bufs=4, space="PSUM") as psG,
            tc.tile_pool(name="psS", bufs=1, space="PSUM") as psS,
        ):
            def load(dram, shape, dt=F16):
                t = consts.tile(list(shape), dt, tag=dram.tensor.name)
                nc.sync.dma_start(t[:], dram[:])
                return t

            zT = load(zT_d, [LAT, BL])
            WihzT = load(WihzT_d, [LAT, G4])
            WihpTa = load(WihpTa_d, [JD + 1, G4])
            WhhT = load(WhhT_d, [128, 4, G4])
            Wa1T = load(Wa1T_d, [128, 4, H])
            Wo1T = load(Wo1T_d, [128, 4, H])
            Wa2T = load(Wa2T_d, [128, 4, JD])
            Wo2T = load(Wo2T_d, [128, 4, OD])
            WzhT = load(WzhT_d, [LAT, H])
            WzcT = load(WzcT_d, [LAT, H])
            bzh = load(bzh_d, [128, 4], F32)
            bzc = load(bzc_d, [1, H])
            ba1 = load(ba1_d, [1, H])
            bo1 = load(bo1_d, [1, H])
            ba2 = load(ba2_d, [JD, 1], F32)
            bo2 = load(bo2_d, [OD, 1], F32)
            WfkT = load(WfkT_d, [JD, FK], F32)
            bfk = load(bfk_d, [FK, 1], F32)
            jrange = load(jrange_d, [JD, 1], F32)
            jmean = load(jmean_d, [JD, 1], F32)
            ones = load(ones_d, [1, BL])
            iden = load(iden_d, [128, 128], F32)

            prevTa = state.tile([JD + 1, BL], F16)
            nc.sync.dma_start(prevTa[:], prevTa_d[:])
            hT = state.tile([128, 4, BL], F16)
            c = state.tile([BL, H], F32)
            h = state.tile([BL, H], F32)
            a1T = state.tile([128, 4, BL], F16)
            o1T = state.tile([128, 4, BL], F16)
            outT = state.tile([32 + OD, TBn], F32)  # rows 0:7 joints^T, rows 32:35 obj^T (32-aligned partition base)

            idq = iden[0:BL, 0:BL]  # 64x64 identity for transposes

            # ---- static PSUM tiles (4 banks; psG holds the other 4) ----
            # tph bank: h-transposes (cols 0:256) + 4 raw-head partial matmuls
            # (cols 256:512) — all single-write matmul groups (start+stop on
            # every instruction), so they can safely share a zero region.
            # tpa bank: a1- AND o1-transposes time-share cols 0:256 (WAR deps
            # order them), obj-head partials on cols 256:512.
            tph = psS.tile([128, H], F32)
            tpa = psS.tile([128, H], F32)
            c0p = psS.tile([BL, H], F32)   # init-only scratch (1 bank)
            rawp = tph[0:JD, 4 * BL : 8 * BL]     # [7, 4*64] partials
            objp = tpa[0:OD, 4 * BL : 8 * BL]     # [3, 4*64] partials

            # ---- init: h0^T (feature-major) and c0 (batch-major) ----
            for m in range(4):
                nc.tensor.matmul(tph[:, ts(m, BL)], WzhT[:, ts(m, 128)], zT[:], start=True, stop=True)
                nc.scalar.activation(hT[:, m, :], tph[:, ts(m, BL)], AF.Identity, bias=bzh[:, m : m + 1])
            nc.tensor.matmul(c0p[:], zT[:], WzcT[:], start=True, stop=False)
            nc.tensor.matmul(c0p[:], ones[:], bzc[:], start=False, stop=True)
            nc.vector.tensor_copy(c[:], c0p[:])

            # ---- recurrence ----
            # Gate banks are partition-packed in pairs: tile gA holds gate f on
            # partitions 0:64 and gate i on 64:128 (so one sigmoid covers both
            # and each tensor_tensor's SBUF inputs share a base partition);
            # gB holds g / o. Upper-half matmuls use PE column group (0,64).
            # Gate pairs partition-packed per PSUM tile: gA = f on partitions
            # 0:64 + i on 64:128 (one sigmoid covers both; f at base 0 pairs
            # with c, i at base 64 pairs with tanh(g) written at base 64);
            # gB = g top / o bottom. Upper halves use PE column group (0,64).
            PAIRS = ((1, 0), (2, 3))

            def z_mms(gp):
                for half, (bt, bb) in enumerate(PAIRS):
                    nc.tensor.matmul(gp[half][0:BL, :], zT[:], WihzT[:, ts(bt, H)], start=True, stop=False, skip_group_check=True)
                    nc.tensor.matmul(gp[half][BL : 2 * BL, :], zT[:], WihzT[:, ts(bb, H)], start=True, stop=False, skip_group_check=True)

            def h_mms(gp, ks):
                for k in ks:
                    lhs = hT[:, k, :]
                    for half, (bt, bb) in enumerate(PAIRS):
                        nc.tensor.matmul(gp[half][0:BL, :], lhs, WhhT[:, k, ts(bt, H)], start=False, stop=False, skip_group_check=True)
                        nc.tensor.matmul(gp[half][BL : 2 * BL, :], lhs, WhhT[:, k, ts(bb, H)], start=False, stop=False, skip_group_check=True)

            def prev_mms(gp):
                for half, (bt, bb) in enumerate(PAIRS):
                    nc.tensor.matmul(gp[half][0:BL, :], prevTa[:], WihpTa[:, ts(bt, H)], start=False, stop=True, skip_group_check=True)
                    nc.tensor.matmul(gp[half][BL : 2 * BL, :], prevTa[:], WihpTa[:, ts(bb, H)], start=False, stop=True, skip_group_check=True)

            def alloc_g(t):
                return [psG.tile([2 * BL, H], F32, tag="g", name=f"g{half}_{t}") for half in range(2)]

            g_cur = alloc_g(0)
            z_mms(g_cur)
            h_mms(g_cur, range(4))
            prev_mms(g_cur)

            for t in range(nsteps):
                last = t + 1 >= nsteps
                if not last:
                    g_next = alloc_g(t + 1)
                    z_mms(g_next)  # PE fills the elementwise window below

                # LSTM cell elementwise (gate order i, f, g, o), processed in
                # two H-halves so the next step's gate matmuls for k=0,1 can
                # start while the second half is still on ACT/DVE.
                gA, gB = g_cur
                sif = acts.tile([2 * BL, H], F32, tag="actw")   # sigmoid(f) top, sigmoid(i) bottom
                tgt = acts.tile([2 * BL, H], F32, tag="actw2")  # tanh(g) on bottom half (base 64, pairs with si)
                so = acts.tile([BL, H], F32, tag="act")
                t1 = acts.tile([BL, H], F32, tag="act")
                t2 = acts.tile([BL, H], F32, tag="act")
                tch = acts.tile([BL, H], F32, tag="act")
                NSPLIT = 2
                HH = H // NSPLIT
                for hh in range(NSPLIT):
                    sl = bass.ds(hh * HH, HH)
                    nc.scalar.activation(sif[:, sl], gA[0 : 2 * BL, sl], AF.Sigmoid)
                    nc.scalar.activation(tgt[BL : 2 * BL, sl], gB[0:BL, sl], AF.Tanh)
                    nc.scalar.activation(so[:, sl], gB[BL : 2 * BL, sl], AF.Sigmoid)
                    nc.gpsimd.tensor_mul(t1[:, sl], sif[0:BL, sl], c[:, sl])
                    nc.vector.tensor_mul(t2[:, sl], sif[BL : 2 * BL, sl], tgt[BL : 2 * BL, sl])
                    nc.vector.tensor_add(c[:, sl], t1[:, sl], t2[:, sl])
                    nc.scalar.activation(tch[:, sl], c[:, sl], AF.Tanh)
                    nc.vector.tensor_mul(h[:, sl], so[:, sl], tch[:, sl])
                    ks = range(4 // NSPLIT * hh, 4 // NSPLIT * (hh + 1))
                    for k in ks:
                        nc.tensor.transpose(tph[:, ts(k, BL)], h[:, ts(k, 128)], idq)
                    nc.vector.tensor_copy(
                        hT[:, ks.start : ks.stop, :].rearrange("p a b -> p (a b)"),
                        tph[:, ks.start * BL : ks.stop * BL],
                    )
                    if not last:
                        h_mms(g_next, ks)

                # head first layers, packed: a1 on partitions 0:64, o1 on 64:128
                m1 = psG.tile([2 * BL, H], F32, tag="m1", bufs=1, name=f"m1_{t}")
                for k in range(4):
                    nc.tensor.matmul(m1[0:BL, :], hT[:, k, :], Wa1T[:, k, :], start=(k == 0), stop=False, skip_group_check=True)
                    nc.tensor.matmul(m1[BL : 2 * BL, :], hT[:, k, :], Wo1T[:, k, :], start=(k == 0), stop=False, skip_group_check=True)
                nc.tensor.matmul(m1[0:BL, :], ones[:], ba1[:], start=False, stop=True, skip_group_check=True)
                nc.tensor.matmul(m1[BL : 2 * BL, :], ones[:], bo1[:], start=False, stop=True, skip_group_check=True)

                # agent head: raw^T = tanh(Wa2 @ relu(a1)^T + ba2)
                # Wa2 contraction is done as 4 side-by-side single-matmul
                # partials + one strided reduce (keeps shared banks free of
                # multi-matmul accumulation groups).
                a1 = acts.tile([BL, H], F32, tag="act")
                nc.scalar.activation(a1[:], m1[0:BL, :], AF.Relu)
                for k in range(4):
                    nc.tensor.transpose(tpa[:, ts(k, BL)], a1[:, ts(k, 128)], idq)
                nc.vector.tensor_copy(a1T[:].rearrange("p a b -> p (a b)"), tpa[:, 0 : 4 * BL])
                for k in range(4):
                    nc.tensor.matmul(rawp[:, ts(k, BL)], Wa2T[:, k, :], a1T[:, k, :], start=True, stop=True)
                raw_red = acts.tile([JD, BL], F32, tag="red", name=f"rr{t}")
                nc.vector.reduce_sum(
                    raw_red[:], rawp.rearrange("p (a b) -> p b a", a=4), axis=mybir.AxisListType.X
                )
                rawt = acts.tile([JD, BL], F32, tag="red", name=f"rt{t}")
                nc.scalar.activation(rawt[:], raw_red[:], AF.Tanh, bias=ba2[:])
                nc.vector.tensor_copy(prevTa[0:JD, :], rawt[:])
                nc.vector.tensor_scalar(
                    outT[0:JD, ts(t, BL)], rawt[:], jrange[:], jmean[:],
                    op0=ALU.mult, op1=ALU.add,
                )

                if not last:
                    prev_mms(g_next)

                # object head — entirely off the critical path; its PE ops run
                # inside the next step's elementwise window. o1-transposes
                # time-share tpa cols 0:256 with the a1-transposes (WAR-ordered).
                o1 = acts.tile([BL, H], F32, tag="act")
                nc.scalar.activation(o1[:], m1[BL : 2 * BL, :], AF.Relu)
                for k in range(4):
                    nc.tensor.transpose(tpa[:, ts(k, BL)], o1[:, ts(k, 128)], idq)
                nc.vector.tensor_copy(o1T[:].rearrange("p a b -> p (a b)"), tpa[:, 0 : 4 * BL])
                for k in range(4):
                    nc.tensor.matmul(objp[:, ts(k, BL)], Wo2T[:, k, :], o1T[:, k, :], start=True, stop=True)
                obj_red = acts.tile([OD, BL], F32, tag="red2", name=f"ob{t}")
                nc.vector.reduce_sum(
                    obj_red[:], objp.rearrange("p (a b) -> p b a", a=4), axis=mybir.AxisListType.X
                )
                nc.scalar.activation(outT[32 : 32 + OD, ts(t, BL)], obj_red[:], AF.Identity, bias=bo2[:])

                if not last:
                    g_cur = g_next

            # ---- FK projection + output DMAs ----
            for nb in range(max(1, TBn // 512)):
                fkp = c0p[0:FK, :]
                fkw = min(512, TBn - nb * 512)
                nc.tensor.matmul(fkp[:, 0:fkw], WfkT[:], outT[0:JD, nb*512:nb*512+fkw], start=True, stop=True)
                ag = acts.tile([FK, 512], F32, tag="act", name=f"ag{nb}")
                nc.scalar.activation(ag[:, 0:fkw], fkp[:, 0:fkw], AF.Identity, bias=bfk[:])
                nc.sync.dma_start(combT_d[0:FK, nb*512:nb*512+fkw], ag[:, 0:fkw])
            nc.sync.dma_start(combT_d[FK : FK + OD, :], outT[32 : 32 + OD, :])
            nc.sync.dma_start(jointT_d[:], outT[0:JD, :])

    nc.compile()
    return nc


def _prep_in_maps(inputs):
    import ml_dtypes
    f32 = lambda a: np.ascontiguousarray(np.asarray(a), dtype=np.float32)
    r = lambda a: np.ascontiguousarray(np.asarray(a, np.float32), dtype=np.float16)

    z = f32(inputs["z"])
    W_ih = f32(inputs["W_ih"])
    W_hh = f32(inputs["W_hh"])
    b_ih = f32(inputs["b_ih"])
    b_hh = f32(inputs["b_hh"])
    W_zh, b_zh = f32(inputs["W_zh"]), f32(inputs["b_zh"])
    W_zc, b_zc = f32(inputs["W_zc"]), f32(inputs["b_zc"])
    Wo1, bo1 = f32(inputs["Wo1"]), f32(inputs["bo1"])
    Wo2, bo2 = f32(inputs["Wo2"]), f32(inputs["bo2"])
    Wa1, ba1 = f32(inputs["Wa1"]), f32(inputs["ba1"])
    Wa2, ba2 = f32(inputs["Wa2"]), f32(inputs["ba2"])
    start_token = f32(inputs["start_token"])
    jl, ju = f32(inputs["joint_lower"]), f32(inputs["joint_upper"])
    W_fk, b_fk = f32(inputs["W_fk"]), f32(inputs["b_fk"])

    jrange = (ju - jl) * 0.5
    jmean = (ju + jl) * 0.5
    prev0 = (start_token - jmean) / jrange

    WihzT = r(W_ih[:, JD:].T)                                   # [128, 2048]
    WihpTa = r(np.concatenate([W_ih[:, :JD].T, (b_ih + b_hh)[None, :]], axis=0))  # [8, 2048]
    WhhT = r(W_hh.T.reshape(4, 128, G4).transpose(1, 0, 2))     # [128, 4, 2048]
    Wa1T = r(Wa1.T.reshape(4, 128, H).transpose(1, 0, 2))
    Wo1T = r(Wo1.T.reshape(4, 128, H).transpose(1, 0, 2))
    Wa2T = r(Wa2.T.reshape(4, 128, JD).transpose(1, 0, 2))
    Wo2T = r(Wo2.T.reshape(4, 128, OD).transpose(1, 0, 2))
    WzhT, WzcT = r(W_zh.T), r(W_zc.T)
    bzh = np.ascontiguousarray(b_zh.reshape(4, 128).T)          # [128, 4]
    shared = {
        "WihzT": WihzT, "WihpTa": WihpTa, "WhhT": WhhT,
        "Wa1T": Wa1T, "Wo1T": Wo1T, "Wa2T": Wa2T, "Wo2T": Wo2T,
        "WzhT": WzhT, "WzcT": WzcT, "bzh": bzh,
        "bzc": r(b_zc[None, :]), "ba1": r(ba1[None, :]), "bo1": r(bo1[None, :]),
        "ba2": ba2[:, None], "bo2": bo2[:, None],
        "WfkT": np.ascontiguousarray(W_fk.T), "bfk": b_fk[:, None],
        "jrange": jrange[:, None], "jmean": jmean[:, None],
        "ones": r(np.ones((1, BL), np.float32)),
        "iden": np.eye(128, dtype=np.float32),
    }
    prevTa0 = r(np.concatenate(
        [np.broadcast_to(prev0[:, None], (JD, BL)), np.ones((1, BL), np.float32)], axis=0))

    in_maps = []
    for cid in range(NCORES):
        zs = z[cid * BL : (cid + 1) * BL]
        m = dict(shared)
        m["zT"] = r(zs.T)
        m["prevTa0"] = prevTa0
        in_maps.append(m)
    return in_maps


def _get_compiled():
    if "nc" not in _CACHE:
        _CACHE["nc"] = _build()
    return _CACHE["nc"]


def run_on_cores(inputs, trace=False, **kw):
    """Compile (cached) + execute; returns (combined, joint, BassKernelResults)."""
    from concourse.bass_utils import run_bass_kernel_spmd

    nc = _get_compiled()
    in_maps = _prep_in_maps(inputs)
    res = run_bass_kernel_spmd(nc, in_maps, list(range(NCORES)), trace=trace, **kw)
    combined = np.empty((B, T, FK + OD), np.float32)
    joint = np.empty((B, T, JD), np.float32)
    for cid in range(NCORES):
        combT = res.results[cid]["combT"]  # [27, T*BL]
        jointT = res.results[cid]["jointT"]  # [7, T*BL]
        sl = slice(cid * BL, (cid + 1) * BL)
        combined[sl] = combT.reshape(FK + OD, T, BL).transpose(2, 1, 0)
        joint[sl] = jointT.reshape(JD, T, BL).transpose(2, 1, 0)
    return combined, joint, res


def kernel(**inputs):
    combined, joint, _ = run_on_cores(inputs)
    return combined, joint
